# revision 1
# baseline (speedup 1.0000x reference)
"""Allegro-style GNN message passing on 8 TRN2 NeuronCores.

Strategy (edges sharded contiguously across 8 cores):
  - Per core 20000 edges, sorted by edge_center, bucketed into 40 blocks of
    128 atoms, each block padded to T_B=5 tiles of 128 edges (E_pad=25600).
  - Feature-major (transposed) activation layout [feat<=128, edges].
  - segment_sum  = per-tile one-hot matmul (PE) accumulating per-block PSUM
    slabs -> bf16 local table [5120, 256] -> AllReduce (bf16) across 8 cores.
  - gather       = per-tile one-hot matmul against the AllReduced table slab.
  - MLPs in bf16 on the TensorEngine with fp32 PSUM accumulation.
Host does layout prep (sort/pad/transpose/cast) + unshard only.
"""

import sys
import math

sys.path.insert(0, "/opt/trn_rl_repo")

import numpy as np
import ml_dtypes

import concourse.bass as bass
import concourse.bacc as bacc
from concourse import tile
import concourse.mybir as mybir
from concourse.bass_utils import run_bass_kernel_spmd

BF = mybir.dt.bfloat16
F32 = mybir.dt.float32
BF_NP = ml_dtypes.bfloat16

# problem constants
E = 160000
N_ATOMS = 5000
C = 64
NS = 128
SCAL_IN = 64
HID = 256
NORM = 1.0 / math.sqrt(32.0)
INV_SQRT3 = 1.0 / math.sqrt(3.0)

N_CORES = 8
E_LOC = E // N_CORES          # 20000
ABLK = 128                    # atoms per block
T_B = 5                       # tiles (of 128 edges) per block
GRP = 4                       # tiles per matmul group (N=512)


def build_graph(n_cores=N_CORES, n_blk=40, t_b=T_B):
    """Build the SPMD Bass graph (same graph runs on every core)."""
    NT = n_blk * t_b              # edge tiles
    E_PAD = NT * 128
    A = n_blk * ABLK              # padded atom count
    assert NT % GRP == 0
    NG = NT // GRP

    nc = bacc.Bacc("TRN2", target_bir_lowering=False, debug=False,
                   num_devices=n_cores)

    # ---- I/O ----
    seT = nc.dram_tensor("seT", [SCAL_IN, E_PAD], BF, kind="ExternalInput")
    tfw = nc.dram_tensor("tfw", [4, 128, E_PAD], BF, kind="ExternalInput")
    t0r = nc.dram_tensor("t0r", [128, E_PAD], BF, kind="ExternalInput")
    basis_pm = nc.dram_tensor("basis_pm", [128, NT, 4], BF, kind="ExternalInput")
    idxq_h = nc.dram_tensor("idxq", [2, 128, NT * 128], BF, kind="ExternalInput")
    iota_part_h = nc.dram_tensor("iota_part", [128, 512], BF, kind="ExternalInput")
    iota_tile_h = nc.dram_tensor("iota_tile", [128, 512], BF, kind="ExternalInput")
    eye_h = nc.dram_tensor("eye", [128, 128], BF, kind="ExternalInput")
    redmat_h = nc.dram_tensor("redmat", [128, 64], BF, kind="ExternalInput")
    wa_h = nc.dram_tensor("wa", [128, 2], F32, kind="ExternalInput")
    wb_h = nc.dram_tensor("wb", [128, 2], F32, kind="ExternalInput")
    wx_h = nc.dram_tensor("wx", [128, 1], F32, kind="ExternalInput")
    wproj_h = nc.dram_tensor("wproj", [64, 256], BF, kind="ExternalInput")
    # lat0: W1 [192,256] -> a[128,256]+b[64,256]; W2/W3 [256,256] -> 2x[128,256]
    l0w1a_h = nc.dram_tensor("l0w1a", [128, 256], BF, kind="ExternalInput")
    l0w1b_h = nc.dram_tensor("l0w1b", [64, 256], BF, kind="ExternalInput")
    l0w2a_h = nc.dram_tensor("l0w2a", [128, 256], BF, kind="ExternalInput")
    l0w2b_h = nc.dram_tensor("l0w2b", [128, 256], BF, kind="ExternalInput")
    l0w3a_h = nc.dram_tensor("l0w3a", [128, 256], BF, kind="ExternalInput")
    l0w3b_h = nc.dram_tensor("l0w3b", [128, 256], BF, kind="ExternalInput")
    # lat1: W1 [320,256] -> 2x[128,256]+[64,256]; W2 [256,256]; W3 [256,128]
    l1w1a_h = nc.dram_tensor("l1w1a", [128, 256], BF, kind="ExternalInput")
    l1w1b_h = nc.dram_tensor("l1w1b", [128, 256], BF, kind="ExternalInput")
    l1w1c_h = nc.dram_tensor("l1w1c", [64, 256], BF, kind="ExternalInput")
    l1w2a_h = nc.dram_tensor("l1w2a", [128, 256], BF, kind="ExternalInput")
    l1w2b_h = nc.dram_tensor("l1w2b", [128, 256], BF, kind="ExternalInput")
    l1w3a_h = nc.dram_tensor("l1w3a", [128, 128], BF, kind="ExternalInput")
    l1w3b_h = nc.dram_tensor("l1w3b", [128, 128], BF, kind="ExternalInput")

    acc0T = nc.dram_tensor("acc0T", [128, E_PAD], BF, kind="ExternalOutput")
    acc1T = nc.dram_tensor("acc1T", [128, E_PAD], BF, kind="ExternalOutput")
    acc2T = nc.dram_tensor("acc2T", [128, E_PAD], BF, kind="ExternalOutput")

    SILU = mybir.ActivationFunctionType.Silu
    MUL = mybir.AluOpType.mult
    ADD = mybir.AluOpType.add
    EQ = mybir.AluOpType.is_equal
    rg = [list(range(n_cores))]

    with tile.TileContext(nc) as tc:
        with tc.tile_pool(name="const", bufs=1) as cpool, \
             tc.tile_pool(name="dram", bufs=1, space="DRAM") as dram, \
             tc.tile_pool(name="qpool", bufs=1) as qpool:

            # ---- constants in SBUF ----
            def cload(h, shape, dt=BF):
                t = cpool.tile(shape, dt, tag=h.name)
                nc.sync.dma_start(out=t[:], in_=h[:])
                return t

            iota_part = cload(iota_part_h, [128, 512])
            iota_tile = cload(iota_tile_h, [128, 512])
            eye = cload(eye_h, [128, 128])
            redmat = cload(redmat_h, [128, 64])
            wa = cload(wa_h, [128, 2], F32)
            wb = cload(wb_h, [128, 2], F32)
            wx = cload(wx_h, [128, 1], F32)
            wproj = cload(wproj_h, [64, 256])
            l0w1a = cload(l0w1a_h, [128, 256])
            l0w1b = cload(l0w1b_h, [64, 256])
            l0w2a = cload(l0w2a_h, [128, 256])
            l0w2b = cload(l0w2b_h, [128, 256])
            l0w3a = cload(l0w3a_h, [128, 256])
            l0w3b = cload(l0w3b_h, [128, 256])
            l1w1a = cload(l1w1a_h, [128, 256])
            l1w1b = cload(l1w1b_h, [128, 256])
            l1w1c = cload(l1w1c_h, [64, 256])
            l1w2a = cload(l1w2a_h, [128, 256])
            l1w2b = cload(l1w2b_h, [128, 256])
            l1w3a = cload(l1w3a_h, [128, 128])
            l1w3b = cload(l1w3b_h, [128, 128])

            # persistent q = [o0(64) | ov0 | ov1 | ov2] feature-major, bf16
            q0 = qpool.tile([128, E_PAD], BF, tag="q0")
            q1 = qpool.tile([128, E_PAD], BF, tag="q1")

            # DRAM bounce buffers for the two AllReduces
            HB0 = n_blk // 2
            HA0 = HB0 * 128
            t1loc = dram.tile([A, 256], BF)
            t1glob_a = dram.tile([HA0, 256], BF, addr_space="Shared")
            t1glob_b = dram.tile([A - HA0, 256], BF, addr_space="Shared")
            t2loc = dram.tile([A, 256], BF)
            t2glob_a = dram.tile([HA0, 256], BF, addr_space="Shared")
            t2glob_b = dram.tile([A - HA0, 256], BF, addr_space="Shared")

            _idxq_memo = {}

            def idxq(pool, g, _phase=[0]):
                key = (pool.name, g)
                if key not in _idxq_memo:
                    t = pool.tile([128, 2, GRP * 128], BF, tag="idxq",
                                  name=f"idxq_{len(_idxq_memo)}")
                    nc.sync.dma_start(
                        out=t[:],
                        in_=idxq_h[:, :, g * GRP * 128:(g + 1) * GRP * 128]
                        .rearrange("c p e -> p c e"))
                    _idxq_memo[key] = t
                return _idxq_memo[key]

            def build_M(pool, g):
                """One-hot scatter matrices for group g: [128e, 4t*128a]."""
                m = pool.tile([128, GRP, 128], BF, tag="M")
                mv = m[:].rearrange("p t a -> p (t a)")
                nc.vector.tensor_tensor(out=mv, in0=idxq(pool, g)[:, 0, :],
                                        in1=iota_tile[:], op=EQ)
                return m

            def build_M2(pool, g):
                """One-hot gather matrices for group g: [128a, 512e]."""
                m2 = pool.tile([128, GRP * 128], BF, tag="M2")
                nc.vector.tensor_tensor(out=m2[:], in0=iota_part[:],
                                        in1=idxq(pool, g)[:, 1, :], op=EQ)
                return m2

            def build_X(pool, spool, envw_sb, scale_col, g):
                """Transpose env_w [128f,512e] to edge-major and expand with
                basis -> X [128e, 4t, 256f]. scale_col: optional [128,1]
                per-channel scale applied before transpose."""
                if scale_col is not None:
                    sc = spool.tile([128, GRP * 128], BF, tag="envws")
                    nc.vector.tensor_scalar_mul(out=sc[:], in0=envw_sb[:],
                                                scalar1=scale_col)
                    envw_sb = sc
                ps = pool.tile([128, GRP * 128], BF, tag="pt")
                for t in range(GRP):
                    nc.tensor.transpose(
                        out=ps[:, t * 128:(t + 1) * 128],
                        in_=envw_sb[:, t * 128:(t + 1) * 128],
                        identity=eye[:])
                x = spool.tile([128, GRP, 256], BF, tag="X")
                bt = spool.tile([128, GRP, 4], BF, tag="bt")
                nc.sync.dma_start(out=bt[:], in_=basis_pm[:, g * GRP:(g + 1) * GRP, :])
                psv = ps[:].rearrange("p (t f) -> p t f", t=GRP)   # [128,4,128]
                # X[:, :, 0:64] = envwT[:, :, 0:64] * b0
                pa = psv[:, :, 0:64]
                ba0 = bt[:, :, 0:1]
                ba = bass.AP(ba0.tensor, ba0.offset,
                             [list(d) for d in ba0.ap[:2]] + [[0, 64]])
                nc.vector.tensor_tensor(out=x[:, :, 0:64], in0=pa, in1=ba, op=MUL)
                # X[:, :, 64:256] = envwT[:, :, 64:128] (x3) * b123
                pb0 = psv[:, :, 64:128]
                pb = bass.AP(pb0.tensor, pb0.offset,
                             [list(pb0.ap[0]), list(pb0.ap[1]), [0, 3],
                              list(pb0.ap[2])])
                bb0 = bt[:, :, 1:4]
                bb = bass.AP(bb0.tensor, bb0.offset,
                             [list(d) for d in bb0.ap[:3]] + [[0, 64]])
                xb = x[:, :, 64:256].rearrange("p t (k f) -> p t k f", k=3)
                nc.vector.tensor_tensor(out=xb, in0=pb, in1=bb, op=MUL)
                return x

            # scatter state shared across groups within a phase
            def make_scatter(pool, spool, tloc):
                state = {"slab": None}

                def scatter_tile(t, m_tile, x_tile, sub):
                    b = t // t_b
                    first = (t % t_b == 0)
                    last = (t % t_b == t_b - 1)
                    if first:
                        state["slab"] = pool.tile([128, 256], F32, tag="slab", name=f"slab_{t}")
                    nc.tensor.matmul(out=state["slab"][:],
                                     lhsT=m_tile[:, sub, :],
                                     rhs=x_tile[:, sub, :],
                                     start=first, stop=last)
                    if last:
                        sl = spool.tile([128, 256], BF, tag="slabsb")
                        nc.vector.tensor_copy(out=sl[:], in_=state["slab"][:])
                        nc.sync.dma_start(out=tloc[b * 128:(b + 1) * 128, :],
                                          in_=sl[:])
                return scatter_tile

            # =================== Phase 1: proj + scatter-1 ===================
            with tc.tile_pool(name="p1sb", bufs=3) as sp, \
                 tc.tile_pool(name="p1ps", bufs=2, space="PSUM") as pp, \
                 tc.tile_pool(name="p1slab", bufs=2, space="PSUM") as slabp:
                scat1 = make_scatter(slabp, sp, t1loc)
                for g in range(NG):
                    sl = slice(g * GRP * 128, (g + 1) * GRP * 128)
                    se = sp.tile([64, GRP * 128], BF, tag="se")
                    nc.sync.dma_start(out=se[:], in_=seT[:, sl])
                    p_a = pp.tile([128, GRP * 128], F32, tag="mm")
                    nc.tensor.matmul(out=p_a[:], lhsT=wproj[:, 0:128],
                                     rhs=se[:], start=True, stop=True)
                    a0 = sp.tile([128, GRP * 128], BF, tag="acc0")
                    nc.scalar.activation(out=a0[:], in_=p_a[:],
                                         func=mybir.ActivationFunctionType.Copy)
                    nc.sync.dma_start(out=acc0T[:, sl], in_=a0[:])
                    p_b = pp.tile([128, GRP * 128], F32, tag="mm")
                    nc.tensor.matmul(out=p_b[:], lhsT=wproj[:, 128:256],
                                     rhs=se[:], start=True, stop=True)
                    ew = sp.tile([128, GRP * 128], BF, tag="envw")
                    nc.scalar.activation(out=ew[:], in_=p_b[:],
                                         func=mybir.ActivationFunctionType.Copy)
                    m = build_M(sp, g)
                    x = build_X(pp, sp, ew, None, g)
                    for s in range(GRP):
                        scat1(g * GRP + s, m, x, s)

            # collective 1 (two halves; first half covers blocks < n_blk//2
            # whose scatters finish early, so the AR overlaps compute)
            HB = n_blk // 2
            HA = HB * 128
            nc.gpsimd.collective_compute(
                "AllReduce", ADD, replica_groups=rg,
                ins=[t1loc[0:HA, :].opt()], outs=[t1glob_a.opt()])
            nc.gpsimd.collective_compute(
                "AllReduce", ADD, replica_groups=rg,
                ins=[t1loc[HA:A, :].opt()], outs=[t1glob_b.opt()])
            T1 = qpool.tile([128, n_blk, 256], BF, tag="T1")
            nc.sync.dma_start(
                out=T1[:, 0:HB, :],
                in_=t1glob_a[:].rearrange("(b p) f -> p b f", p=128))
            nc.sync.dma_start(
                out=T1[:, HB:n_blk, :],
                in_=t1glob_b[:].rearrange("(b p) f -> p b f", p=128))

            # =================== Phase 3: gather-1, TP0, mlp0, scatter-2 =====
            with tc.tile_pool(name="p3sb", bufs=2) as sp, \
                 tc.tile_pool(name="p3ps", bufs=3, space="PSUM") as pp, \
                 tc.tile_pool(name="p3red", bufs=1, space="PSUM") as redp, \
                 tc.tile_pool(name="p3pt", bufs=1, space="PSUM") as ppt, \
                 tc.tile_pool(name="p3env", bufs=2, space="PSUM") as ep, \
                 tc.tile_pool(name="p3slab", bufs=1, space="PSUM") as slabp:
                scat2 = make_scatter(slabp, sp, t2loc)
                for g in range(NG):
                    sl = slice(g * GRP * 128, (g + 1) * GRP * 128)
                    m2 = build_M2(sp, g)
                    pe0 = ep.tile([128, GRP * 128], F32, tag="env")
                    pe1 = ep.tile([128, GRP * 128], F32, tag="env")
                    t = 0
                    while t < GRP:
                        b = (g * GRP + t) // t_b
                        n = 1
                        while t + n < GRP and (g * GRP + t + n) // t_b == b:
                            n += 1
                        ts = slice(t * 128, (t + n) * 128)
                        nc.tensor.matmul(out=pe0[:, ts], lhsT=T1[:, b, 0:128],
                                         rhs=m2[:, ts], start=True, stop=True)
                        nc.tensor.matmul(out=pe1[:, ts], lhsT=T1[:, b, 128:256],
                                         rhs=m2[:, ts], start=True, stop=True)
                        t += n
                    e0 = sp.tile([128, GRP * 128], BF, tag="e0")
                    e1 = sp.tile([128, GRP * 128], BF, tag="e1")
                    nc.scalar.activation(out=e0[:], in_=pe0[:],
                                         func=mybir.ActivationFunctionType.Copy)
                    nc.scalar.activation(out=e1[:], in_=pe1[:],
                                         func=mybir.ActivationFunctionType.Copy)
                    tfq = sp.tile([128, 4, GRP * 128], BF, tag="tfq")
                    nc.sync.dma_start(out=tfq[:], in_=tfw[:, :, sl].rearrange("c p e -> p c e"))
                    t0b = sp.tile([128, GRP * 128], BF, tag="t0b")
                    nc.sync.dma_start(out=t0b[:], in_=t0r[:, sl])
                    tfa0 = tfq[:, 0, :]
                    tfa1 = tfq[:, 1, :]
                    tfb0 = tfq[:, 2, :]
                    tfb1 = tfq[:, 3, :]
                    # TP0 (feature-major, batched over the group).
                    # Two-input SBUF ops must share base partition; outputs
                    # may be shifted, so route all cross-half products through
                    # base-0 temp tiles / shifted helper copies.
                    GW = GRP * 128
# full-height products; o0 reduced on PE via redmat
                    ma = sp.tile([128, GW], BF, tag="ma")
                    mb = sp.tile([128, GW], BF, tag="mb")
                    nc.vector.tensor_tensor(out=ma[:], in0=tfa0[:, :], in1=e0[:], op=MUL)
                    nc.vector.tensor_tensor(out=mb[:], in0=tfa1[:, :], in1=e1[:], op=MUL)
                    po0 = redp.tile([64, GW], F32, tag="po0")
                    nc.tensor.matmul(out=po0[:], lhsT=redmat[:], rhs=ma[:],
                                     start=True, stop=False)
                    nc.tensor.matmul(out=po0[:], lhsT=redmat[:], rhs=mb[:],
                                     start=False, stop=True)
                    nc.scalar.activation(out=q0[0:64, sl], in_=po0[:],
                                         func=mybir.ActivationFunctionType.Copy)
                    # helper: sa on both halves (cheap bf16 DVE copies)
                    hi = sp.tile([128, GW], BF, tag="hi")
                    nc.vector.tensor_copy(out=hi[0:64, :], in_=e0[0:64, :])
                    nc.vector.tensor_copy(out=hi[64:128, :], in_=e0[0:64, :])
                    # ov_k = sa*(wb.tv_k) + va_k*(wb.t0), full-height products
                    pa0 = sp.tile([128, GW], BF, tag="pa0")
                    pa1 = sp.tile([128, GW], BF, tag="pa1")
                    pb0 = sp.tile([128, GW], BF, tag="pb0")
                    pb1 = sp.tile([128, GW], BF, tag="pb1")
                    nc.vector.tensor_tensor(out=pa0[:], in0=hi[:], in1=tfb0[:, :], op=MUL)
                    nc.vector.tensor_tensor(out=pa1[:], in0=hi[:], in1=tfb1[:, :], op=MUL)
                    nc.vector.tensor_tensor(out=pb0[:], in0=e0[:], in1=t0b[:], op=MUL)
                    nc.vector.tensor_tensor(out=pb1[:], in0=e1[:], in1=t0b[:], op=MUL)
                    nc.vector.tensor_add(out=q0[64:128, sl], in0=pa0[64:128, :],
                                         in1=pb0[64:128, :])
                    nc.vector.tensor_add(out=q1[0:64, sl], in0=pa1[0:64, :],
                                         in1=pb1[0:64, :])
                    nc.vector.tensor_add(out=q1[64:128, sl], in0=pa1[64:128, :],
                                         in1=pb1[64:128, :])
                    # mlp0: x = [acc0 ; o0]
                    ac0 = sp.tile([128, GRP * 128], BF, tag="ac0")
                    nc.sync.dma_start(out=ac0[:], in_=acc0T[:, sl])
                    h1a = sp.tile([128, GRP * 128], BF, tag="h1a")
                    h1b = sp.tile([128, GRP * 128], BF, tag="h1b")
                    for mbi, (w1hi, w1lo, hout) in enumerate(
                            [(l0w1a[:, 0:128], l0w1b[:, 0:128], h1a),
                             (l0w1a[:, 128:256], l0w1b[:, 128:256], h1b)]):
                        ph = pp.tile([128, GRP * 128], F32, tag="mm")
                        nc.tensor.matmul(out=ph[:], lhsT=w1hi, rhs=ac0[:],
                                         start=True, stop=False)
                        nc.tensor.matmul(out=ph[:], lhsT=w1lo, rhs=q0[0:64, sl],
                                         start=False, stop=True)
                        nc.scalar.activation(out=hout[:], in_=ph[:], func=SILU)
                    h2a = sp.tile([128, GRP * 128], BF, tag="h2a")
                    h2b = sp.tile([128, GRP * 128], BF, tag="h2b")
                    for mbi, (wa2, wb2, hout) in enumerate(
                            [(l0w2a[:, 0:128], l0w2b[:, 0:128], h2a),
                             (l0w2a[:, 128:256], l0w2b[:, 128:256], h2b)]):
                        ph = pp.tile([128, GRP * 128], F32, tag="mm")
                        nc.tensor.matmul(out=ph[:], lhsT=wa2, rhs=h1a[:],
                                         start=True, stop=False)
                        nc.tensor.matmul(out=ph[:], lhsT=wb2, rhs=h1b[:],
                                         start=False, stop=True)
                        nc.scalar.activation(out=hout[:], in_=ph[:], func=SILU)
                    # lat = h2 @ W3 : M-block0 = acc1, M-block1 = env_w2
                    pl_a = pp.tile([128, GRP * 128], F32, tag="mm")
                    nc.tensor.matmul(out=pl_a[:], lhsT=l0w3a[:, 0:128], rhs=h2a[:],
                                     start=True, stop=False)
                    nc.tensor.matmul(out=pl_a[:], lhsT=l0w3b[:, 0:128], rhs=h2b[:],
                                     start=False, stop=True)
                    a1sb = sp.tile([128, GRP * 128], BF, tag="a1sb")
                    nc.vector.tensor_copy(out=a1sb[:], in_=pl_a[:])
                    nc.sync.dma_start(out=acc1T[:, sl], in_=a1sb[:])
                    pl_b = pp.tile([128, GRP * 128], F32, tag="mm")
                    nc.tensor.matmul(out=pl_b[:], lhsT=l0w3a[:, 128:256], rhs=h2a[:],
                                     start=True, stop=False)
                    nc.tensor.matmul(out=pl_b[:], lhsT=l0w3b[:, 128:256], rhs=h2b[:],
                                     start=False, stop=True)
                    ew2 = sp.tile([128, GRP * 128], BF, tag="envw")
                    nc.scalar.activation(out=ew2[:], in_=pl_b[:],
                                         func=mybir.ActivationFunctionType.Copy)
                    m = build_M(sp, g)
                    x2 = build_X(ppt, sp, ew2, None, g)
                    for s in range(GRP):
                        scat2(g * GRP + s, m, x2, s)

            # collective 2 (halved like collective 1)
            nc.gpsimd.collective_compute(
                "AllReduce", ADD, replica_groups=rg,
                ins=[t2loc[0:HA, :].opt()], outs=[t2glob_a.opt()])
            nc.gpsimd.collective_compute(
                "AllReduce", ADD, replica_groups=rg,
                ins=[t2loc[HA:A, :].opt()], outs=[t2glob_b.opt()])
            T2 = qpool.tile([128, n_blk, 256], BF, tag="T2")
            nc.sync.dma_start(
                out=T2[:, 0:HB, :],
                in_=t2glob_a[:].rearrange("(b p) f -> p b f", p=128))
            nc.sync.dma_start(
                out=T2[:, HB:n_blk, :],
                in_=t2glob_b[:].rearrange("(b p) f -> p b f", p=128))

            # =================== Phase 5: gather-2, TP1, mlp1 ================
            with tc.tile_pool(name="p5sb", bufs=2) as sp, \
                 tc.tile_pool(name="p5ps", bufs=3, space="PSUM") as pp, \
                 tc.tile_pool(name="p5red", bufs=2, space="PSUM") as redp, \
                 tc.tile_pool(name="p5env", bufs=2, space="PSUM") as ep:
                for g in range(NG):
                    sl = slice(g * GRP * 128, (g + 1) * GRP * 128)
                    m2 = build_M2(sp, g)
                    pe0 = ep.tile([128, GRP * 128], F32, tag="env")
                    pe1 = ep.tile([128, GRP * 128], F32, tag="env")
                    t = 0
                    while t < GRP:
                        b = (g * GRP + t) // t_b
                        n = 1
                        while t + n < GRP and (g * GRP + t + n) // t_b == b:
                            n += 1
                        ts = slice(t * 128, (t + n) * 128)
                        nc.tensor.matmul(out=pe0[:, ts], lhsT=T2[:, b, 0:128],
                                         rhs=m2[:, ts], start=True, stop=True)
                        nc.tensor.matmul(out=pe1[:, ts], lhsT=T2[:, b, 128:256],
                                         rhs=m2[:, ts], start=True, stop=True)
                        t += n
                    e0 = sp.tile([128, GRP * 128], BF, tag="e0")
                    e1 = sp.tile([128, GRP * 128], BF, tag="e1")
                    nc.vector.tensor_copy(out=e0[:], in_=pe0[:])
                    nc.vector.tensor_copy(out=e1[:], in_=pe1[:])
                    # o_last = sum_blk env2 . q   (w10/w11 already folded into X2)
                    GW = GRP * 128
                    pa = sp.tile([128, GW], BF, tag="pa")
                    pb = sp.tile([128, GW], BF, tag="pb")
                    nc.vector.tensor_tensor(out=pa[:], in0=e0[:], in1=q0[:, sl], op=MUL)
                    nc.vector.tensor_tensor(out=pb[:], in0=e1[:], in1=q1[:, sl], op=MUL)
                    pol = redp.tile([64, GW], F32, tag="pol")
                    nc.tensor.matmul(out=pol[:], lhsT=redmat[:], rhs=pa[:],
                                     start=True, stop=False)
                    nc.tensor.matmul(out=pol[:], lhsT=redmat[:], rhs=pb[:],
                                     start=False, stop=True)
                    ol = sp.tile([64, GW], BF, tag="ol")
                    nc.scalar.activation(out=ol[:], in_=pol[:],
                                         func=mybir.ActivationFunctionType.Copy)
                    # mlp1: x = [acc0 ; acc1 ; o_last]
                    ac0 = sp.tile([128, GRP * 128], BF, tag="ac0")
                    ac1 = sp.tile([128, GRP * 128], BF, tag="ac1")
                    nc.sync.dma_start(out=ac0[:], in_=acc0T[:, sl])
                    nc.sync.dma_start(out=ac1[:], in_=acc1T[:, sl])
                    h1a = sp.tile([128, GRP * 128], BF, tag="h1a")
                    h1b = sp.tile([128, GRP * 128], BF, tag="h1b")
                    for mbi, hout in enumerate([h1a, h1b]):
                        msl = slice(mbi * 128, (mbi + 1) * 128)
                        ph = pp.tile([128, GRP * 128], F32, tag="mm")
                        nc.tensor.matmul(out=ph[:], lhsT=l1w1a[:, msl], rhs=ac0[:],
                                         start=True, stop=False)
                        nc.tensor.matmul(out=ph[:], lhsT=l1w1b[:, msl], rhs=ac1[:],
                                         start=False, stop=False)
                        nc.tensor.matmul(out=ph[:], lhsT=l1w1c[:, msl], rhs=ol[:],
                                         start=False, stop=True)
                        nc.scalar.activation(out=hout[:], in_=ph[:], func=SILU)
                    h2a = sp.tile([128, GRP * 128], BF, tag="h2a")
                    h2b = sp.tile([128, GRP * 128], BF, tag="h2b")
                    for mbi, hout in enumerate([h2a, h2b]):
                        msl = slice(mbi * 128, (mbi + 1) * 128)
                        ph = pp.tile([128, GRP * 128], F32, tag="mm")
                        nc.tensor.matmul(out=ph[:], lhsT=l1w2a[:, msl], rhs=h1a[:],
                                         start=True, stop=False)
                        nc.tensor.matmul(out=ph[:], lhsT=l1w2b[:, msl], rhs=h1b[:],
                                         start=False, stop=True)
                        nc.scalar.activation(out=hout[:], in_=ph[:], func=SILU)
                    ph = pp.tile([128, GRP * 128], F32, tag="mm")
                    nc.tensor.matmul(out=ph[:], lhsT=l1w3a[:], rhs=h2a[:],
                                     start=True, stop=False)
                    nc.tensor.matmul(out=ph[:], lhsT=l1w3b[:], rhs=h2b[:],
                                     start=False, stop=True)
                    a2sb = sp.tile([128, GRP * 128], BF, tag="a2sb")
                    nc.vector.tensor_copy(out=a2sb[:], in_=ph[:])
                    nc.sync.dma_start(out=acc2T[:, sl], in_=a2sb[:])

    nc.compile()
    return nc, dict(NT=NT, E_PAD=E_PAD, A=A, n_blk=n_blk, t_b=t_b)


# =====================================================================
# Host side
# =====================================================================

def _prep_core(centers_l, se_l, tf_l, basis_l, n_blk, t_b):
    """Sort/pad one core's edges into the block-tile layout."""
    NT = n_blk * t_b
    E_PAD = NT * 128
    order = np.argsort(centers_l, kind="stable")
    bid = centers_l[order] // ABLK
    perm = np.full(E_PAD, -1, np.int64)
    for b in range(n_blk):
        run = order[bid == b]
        assert len(run) <= t_b * 128, f"block {b} overflow: {len(run)}"
        perm[b * t_b * 128: b * t_b * 128 + len(run)] = run
    valid = perm >= 0
    psafe = np.where(valid, perm, 0)

    idxrel = np.where(
        valid, centers_l[psafe] - ABLK * (np.arange(E_PAD) // (t_b * 128)),
        -1).astype(np.float32)

    se_pad = se_l[psafe] * valid[:, None]          # [E_PAD, 64]
    tf_pad = tf_l[psafe] * valid[:, None, None]    # [E_PAD, 64, 4]
    basis_pad = basis_l[psafe] * valid[:, None]    # [E_PAD, 4]

    seT = np.ascontiguousarray(se_pad.T).astype(BF_NP)
    # tfT chunks: row r of chunk c = component/channel feature layout
    # feature f in [0,256): block k=f//64 (0=t0, k=1..3 tv_{k-1}), channel f%64
    tf_cm = np.ascontiguousarray(
        tf_pad.transpose(2, 1, 0)).reshape(4 * 64, E_PAD).astype(np.float32)
    # rows f = 64*comp+chan; chunks: c0 = rows 0:128 (t0|tv0), c1 = rows 128:256
    wa_v, wb_v = _WAB["wa"], _WAB["wb"]
    tfa = tf_cm * wa_v[:, None]
    tfb = tf_cm * wb_v[:, None]
    tfw = np.ascontiguousarray(np.stack(
        [tfa[0:128], tfa[128:256], tfb[0:128], tfb[128:256]])).astype(BF_NP)
    t0r = np.ascontiguousarray(
        np.tile(tfb[0:64], (2, 1))).astype(BF_NP)
    basis_pm = np.ascontiguousarray(
        basis_pad.reshape(NT, 128, 4).transpose(1, 0, 2)).astype(BF_NP)
    idxrel_col = np.ascontiguousarray(
        idxrel.reshape(NT, 128).T).astype(BF_NP)           # [128, NT]
    idxrel_cr = np.ascontiguousarray(
        np.repeat(idxrel_col[:, :, None], 128, axis=2).reshape(128, -1))
    idxq = np.ascontiguousarray(np.stack([
        idxrel_cr,
        np.tile(idxrel.astype(BF_NP)[None, :], (128, 1))]).transpose(0, 1, 2))
    return dict(seT=seT, tfw=tfw, t0r=t0r, basis_pm=basis_pm, idxq=idxq), perm


_WAB = {}


def _weights_maps(W_proj, w_tp0, w_tp1, l0, l1):
    bf = lambda a: np.ascontiguousarray(a).astype(BF_NP)
    wa = np.concatenate([w_tp0[0] * NORM,
                         np.tile(w_tp0[1] * NORM * INV_SQRT3, 3)])  # [256]
    wb = np.concatenate([w_tp0[3] * NORM,
                         np.tile(w_tp0[2] * NORM, 3)])              # [256]
    wx = np.concatenate([w_tp1[0] * NORM,
                         w_tp1[1] * NORM * INV_SQRT3])              # [128]
    _WAB["wa"] = wa.astype(np.float32)
    _WAB["wb"] = wb.astype(np.float32)
    l0w1, l0w2, l0w3 = l0
    l0w3 = l0w3.copy()
    l0w3[:, 128:256] = l0w3[:, 128:256] * wx[None, :]
    l1w1, l1w2, l1w3 = l1
    return {
        "iota_part": bf(np.repeat(np.arange(128, dtype=np.float32)[:, None], 512, 1)),
        "iota_tile": bf(np.tile(np.arange(128, dtype=np.float32)[None, :], (128, 4))),
        "eye": bf(np.eye(128, dtype=np.float32)),
        "redmat": bf(np.tile(np.eye(64, dtype=np.float32), (2, 1))),
        "wa": np.ascontiguousarray(wa.reshape(2, 128).T, np.float32),
        "wb": np.ascontiguousarray(wb.reshape(2, 128).T, np.float32),
        "wx": np.ascontiguousarray(wx[:, None], np.float32),
        "wproj": bf(W_proj),
        "l0w1a": bf(l0w1[0:128]), "l0w1b": bf(l0w1[128:192]),
        "l0w2a": bf(l0w2[0:128]), "l0w2b": bf(l0w2[128:256]),
        "l0w3a": bf(l0w3[0:128]), "l0w3b": bf(l0w3[128:256]),
        "l1w1a": bf(l1w1[0:128]), "l1w1b": bf(l1w1[128:256]),
        "l1w1c": bf(l1w1[256:320]),
        "l1w2a": bf(l1w2[0:128]), "l1w2b": bf(l1w2[128:256]),
        "l1w3a": bf(l1w3[0:128]), "l1w3b": bf(l1w3[128:256]),
    }


_CACHE = {}


def kernel(edge_index, num_atoms, tensor_basis, tensor_features, scalar_embed,
           W_proj, w_tp0, w_tp1,
           lat0_W1, lat0_W2, lat0_W3, lat1_W1, lat1_W2, lat1_W3,
           _trace=False):
    if "nc" not in _CACHE:
        _CACHE["nc"], _CACHE["meta"] = build_graph()
    nc, meta = _CACHE["nc"], _CACHE["meta"]
    n_blk, t_b, E_PAD = meta["n_blk"], meta["t_b"], meta["E_PAD"]

    edge_index = np.asarray(edge_index)
    centers = edge_index[0]
    tb = np.asarray(tensor_basis, np.float32)
    tf = np.asarray(tensor_features, np.float32)
    se = np.asarray(scalar_embed, np.float32)

    wmaps = _weights_maps(
        np.asarray(W_proj, np.float32), np.asarray(w_tp0, np.float32),
        np.asarray(w_tp1, np.float32),
        (np.asarray(lat0_W1, np.float32), np.asarray(lat0_W2, np.float32),
         np.asarray(lat0_W3, np.float32)),
        (np.asarray(lat1_W1, np.float32), np.asarray(lat1_W2, np.float32),
         np.asarray(lat1_W3, np.float32)))

    in_maps, perms = [], []
    for c in range(N_CORES):
        s = slice(c * E_LOC, (c + 1) * E_LOC)
        m, perm = _prep_core(centers[s], se[s], tf[s], tb[s], n_blk, t_b)
        m.update(wmaps)
        in_maps.append(m)
        perms.append(perm)

    res = run_bass_kernel_spmd(nc, in_maps, core_ids=list(range(N_CORES)),
                               trace=_trace)
    out = np.empty((E, NS * 3), np.float32)
    for c in range(N_CORES):
        r = res.results[c]
        op = np.concatenate(
            [np.asarray(r["acc0T"]).astype(np.float32).T,
             np.asarray(r["acc1T"]).astype(np.float32).T,
             np.asarray(r["acc2T"]).astype(np.float32).T], axis=1)  # [E_PAD,384]
        perm = perms[c]
        valid = perm >= 0
        out[c * E_LOC + perm[valid]] = op[valid]
    if _trace:
        kernel.last_exec_time_ns = res.exec_time_ns
    return out



# revision 10
# speedup vs baseline: 1.2407x; 1.2407x over previous
"""Allegro-style GNN message passing on 8 TRN2 NeuronCores.

Strategy (edges partitioned by CENTER atom -> no collectives at all):
  - Both the segment_sum and the gather key on edge_index[0], so assigning
    each edge to the core that owns its center atom makes the env
    scatter/gather purely core-local.  The per-atom tables (640 atoms/core)
    live in SBUF; the AllReduces of an edge-sliced layout disappear.
  - Atoms are bin-packed into 8 cores x 5 blocks (<=128 atoms and <=4096
    edges per block), so the tile->block map is the compile-time constant
    t//32 in the shared SPMD graph.  E_PAD = 5*4096 = 20480 per core.
  - Per group of 512 edges activations are feature-major [feat, edge];
    env_w is produced edge-major directly by transposed matmuls (no PE
    transposes / evictions), the one-hot scatter/gather runs on the PE,
    and the tensor-product scalings (w_tp0 * NORM) are folded on-chip via
    fused scalar_tensor_tensor ops.
  - Per-group inputs ship in packed per-phase DRAM blobs -> one large
    contiguous DMA per group/phase instead of many strided descriptors.
"""

import sys
import math

sys.path.insert(0, "/opt/trn_rl_repo")

import numpy as np
import ml_dtypes

import concourse.bass as bass
import concourse.bacc as bacc
from concourse import tile
import concourse.mybir as mybir
from concourse.bass_utils import run_bass_kernel_spmd

BF = mybir.dt.bfloat16
F32 = mybir.dt.float32
BF_NP = ml_dtypes.bfloat16

# problem constants
E = 160000
N_ATOMS = 5000
C = 64
NS = 128
SCAL_IN = 64
HID = 256
NORM = 1.0 / math.sqrt(32.0)
INV_SQRT3 = 1.0 / math.sqrt(3.0)

N_CORES = 8
N_BLK = 5                     # atom blocks (of 128) per core
T_B = 32                      # tiles (of 128 edges) per block
GRP = 4                       # tiles per matmul group (free dim 512)
NT = N_BLK * T_B              # 160 edge tiles per core
E_PAD = NT * 128              # 20480 padded edges per core
NG = NT // GRP                # 40 groups
GPB = T_B // GRP              # 8 groups per block

GW = GRP * 128                # 512
# blob1: [se | idxM | bt]
B1_SE, B1_IDXM, B1_BT = 0, GW, 2 * GW
B1W = 2 * GW + GRP * 4
# blob3: [tf0 | tf1 | t0b | idxM | idxM2 | bt]
B3_TF0, B3_TF1, B3_T0B, B3_IDXM, B3_IDXM2, B3_BT = (
    0, GW, 2 * GW, 3 * GW, 4 * GW, 5 * GW)
B3W = 5 * GW + GRP * 4
# blob5: [idxM2]
B5W = GW


def build_graph(n_cores=N_CORES):
    nc = bacc.Bacc("TRN2", target_bir_lowering=False, debug=False,
                   num_devices=n_cores)

    blob1_h = nc.dram_tensor("blob1", [128, NG, B1W], BF, kind="ExternalInput")
    blob3_h = nc.dram_tensor("blob3", [128, NG, B3W], BF, kind="ExternalInput")
    blob5_h = nc.dram_tensor("blob5", [128, NG, B5W], BF, kind="ExternalInput")
    iota_part_h = nc.dram_tensor("iota_part", [128, GW], BF, kind="ExternalInput")
    iota_tile_h = nc.dram_tensor("iota_tile", [128, GW], BF, kind="ExternalInput")
    redmat_h = nc.dram_tensor("redmat", [128, 64], BF, kind="ExternalInput")
    wa_h = nc.dram_tensor("wa", [128, 2], F32, kind="ExternalInput")
    wb_h = nc.dram_tensor("wb", [128, 2], F32, kind="ExternalInput")
    wt0_h = nc.dram_tensor("wt0", [128, 1], F32, kind="ExternalInput")
    wproj_h = nc.dram_tensor("wproj", [64, 256], BF, kind="ExternalInput")
    l0w1a_h = nc.dram_tensor("l0w1a", [128, 256], BF, kind="ExternalInput")
    l0w1b_h = nc.dram_tensor("l0w1b", [64, 256], BF, kind="ExternalInput")
    l0w2a_h = nc.dram_tensor("l0w2a", [128, 256], BF, kind="ExternalInput")
    l0w2b_h = nc.dram_tensor("l0w2b", [128, 256], BF, kind="ExternalInput")
    l0w3a_h = nc.dram_tensor("l0w3a", [128, 256], BF, kind="ExternalInput")
    l0w3b_h = nc.dram_tensor("l0w3b", [128, 256], BF, kind="ExternalInput")
    l1w1a_h = nc.dram_tensor("l1w1a", [128, 256], BF, kind="ExternalInput")
    l1w1b_h = nc.dram_tensor("l1w1b", [128, 256], BF, kind="ExternalInput")
    l1w1c_h = nc.dram_tensor("l1w1c", [64, 256], BF, kind="ExternalInput")
    l1w2a_h = nc.dram_tensor("l1w2a", [128, 256], BF, kind="ExternalInput")
    l1w2b_h = nc.dram_tensor("l1w2b", [128, 256], BF, kind="ExternalInput")
    l1w3a_h = nc.dram_tensor("l1w3a", [128, 128], BF, kind="ExternalInput")
    l1w3b_h = nc.dram_tensor("l1w3b", [128, 128], BF, kind="ExternalInput")

    acc0T = nc.dram_tensor("acc0T", [128, E_PAD], BF, kind="ExternalOutput")
    acc1T = nc.dram_tensor("acc1T", [128, E_PAD], BF, kind="ExternalOutput")
    acc2T = nc.dram_tensor("acc2T", [128, E_PAD], BF, kind="ExternalOutput")

    SILU = mybir.ActivationFunctionType.Silu
    COPY = mybir.ActivationFunctionType.Copy
    MUL = mybir.AluOpType.mult
    EQ = mybir.AluOpType.is_equal

    def bcast(ap, dims):
        return bass.AP(ap.tensor, ap.offset, dims)

    with tile.TileContext(nc) as tc:
        with tc.tile_pool(name="const", bufs=1) as cpool, \
             tc.tile_pool(name="qpool", bufs=1) as qpool:

            def cload(h, shape, dt=BF):
                t = cpool.tile(shape, dt, tag=h.name)
                nc.sync.dma_start(out=t[:], in_=h[:])
                return t

            iota_part = cload(iota_part_h, [128, GW])
            iota_tile = cload(iota_tile_h, [128, GW])
            redmat = cload(redmat_h, [128, 64])
            wa = cload(wa_h, [128, 2], F32)
            wb = cload(wb_h, [128, 2], F32)
            wt0 = cload(wt0_h, [128, 1], F32)
            wproj = cload(wproj_h, [64, 256])
            l0w1a = cload(l0w1a_h, [128, 256])
            l0w1b = cload(l0w1b_h, [64, 256])
            l0w2a = cload(l0w2a_h, [128, 256])
            l0w2b = cload(l0w2b_h, [128, 256])
            l0w3a = cload(l0w3a_h, [128, 256])
            l0w3b = cload(l0w3b_h, [128, 256])
            l1w1a = cload(l1w1a_h, [128, 256])
            l1w1b = cload(l1w1b_h, [128, 256])
            l1w1c = cload(l1w1c_h, [64, 256])
            l1w2a = cload(l1w2a_h, [128, 256])
            l1w2b = cload(l1w2b_h, [128, 256])
            l1w3a = cload(l1w3a_h, [128, 128])
            l1w3b = cload(l1w3b_h, [128, 128])

            # persistent feature-major activations
            q0 = qpool.tile([128, E_PAD], BF, tag="q0")      # [o0 | ov0]
            q1 = qpool.tile([128, E_PAD], BF, tag="q1")      # [ov1 | ov2]
            T1 = qpool.tile([128, N_BLK, 256], BF, tag="T1")
            T2 = qpool.tile([128, N_BLK, 256], BF, tag="T2")

            def build_X(spool, envps, bt, tag):
                """X [128e, GRP, 256f] from edge-major env_w (PSUM) * basis.

                bt: [128, GRP, 4] basis tile view (edge-major)."""
                x = spool.tile([128, GRP, 256], BF, tag=tag)
                pa = envps[:, :, 0:64]
                ba0 = bt[:, :, 0:1]
                ba = bcast(ba0, [list(d) for d in ba0.ap[:2]] + [[0, 64]])
                nc.vector.tensor_tensor(out=x[:, :, 0:64], in0=pa, in1=ba, op=MUL)
                pb0 = envps[:, :, 64:128]
                pb = bcast(pb0, [list(pb0.ap[0]), list(pb0.ap[1]), [0, 3],
                                 list(pb0.ap[2])])
                bb0 = bt[:, :, 1:4]
                bb = bcast(bb0, [list(d) for d in bb0.ap[:3]] + [[0, 64]])
                xb = x[:, :, 64:256].rearrange("p t (k f) -> p t k f", k=3)
                nc.vector.tensor_tensor(out=xb, in0=pb, in1=bb, op=MUL)
                return x

            def build_M(spool, idx_ap):
                m = spool.tile([128, GRP, 128], BF, tag="M")
                mv = m[:].rearrange("p t a -> p (t a)")
                nc.vector.tensor_tensor(out=mv, in0=idx_ap,
                                        in1=iota_tile[:], op=EQ)
                return m

            def build_M2(spool, idx_ap):
                m2 = spool.tile([128, GW], BF, tag="M2")
                nc.vector.tensor_tensor(out=m2[:], in0=iota_part[:],
                                        in1=idx_ap, op=EQ)
                return m2

            # =================== Phase 1: proj + scatter-1 ===================
            with tc.tile_pool(name="p1sb", bufs=3) as sp, \
                 tc.tile_pool(name="p1ps", bufs=2, space="PSUM") as pp, \
                 tc.tile_pool(name="p1env", bufs=2, space="PSUM") as envp, \
                 tc.tile_pool(name="p1slab", bufs=1, space="PSUM") as slabp:
                for g in range(NG):
                    bk = g // GPB
                    sl = slice(g * GW, (g + 1) * GW)
                    b1 = sp.tile([128, B1W], BF, tag="b1")
                    nc.sync.dma_start(out=b1[:], in_=blob1_h[:, g, :])
                    se = b1[0:64, B1_SE:B1_SE + GW]
                    # acc0 = se @ Wproj[:, :128]  (feature-major)
                    pacc = pp.tile([128, GW], F32, tag="mm")
                    nc.tensor.matmul(out=pacc[:], lhsT=wproj[:, 0:128],
                                     rhs=se, start=True, stop=True)
                    a0t = sp.tile([128, GW], BF, tag="a0")
                    nc.scalar.activation(out=a0t[:], in_=pacc[:], func=COPY)
                    nc.sync.dma_start(out=acc0T[:, sl], in_=a0t[:])
                    # env_w edge-major: [128e, t, 128f]
                    envps = envp.tile([128, GRP, 128], F32, tag="env")
                    for t in range(GRP):
                        nc.tensor.matmul(
                            out=envps[:, t, :],
                            lhsT=se[:, t * 128:(t + 1) * 128],
                            rhs=wproj[:, 128:256], start=True, stop=True)
                    bt = b1[:, B1_BT:B1_BT + GRP * 4].rearrange(
                        "p (t c) -> p t c", t=GRP)
                    x = build_X(sp, envps, bt, "X")
                    m = build_M(sp, b1[:, B1_IDXM:B1_IDXM + GW])
                    if g % GPB == 0:
                        slabs = slabp.tile([128, 256], F32, tag="slab",
                                           name=f"slab1_{bk}")
                    for s in range(GRP):
                        nc.tensor.matmul(out=slabs[:],
                                         lhsT=m[:, s, :], rhs=x[:, s, :],
                                         start=(g % GPB == 0 and s == 0),
                                         stop=(g % GPB == GPB - 1 and s == GRP - 1))
                    if g % GPB == GPB - 1:
                        nc.scalar.activation(out=T1[:, bk, :], in_=slabs[:],
                                             func=COPY)

            # ========= Phase 3: gather-1, TP0, mlp0, scatter-2 ===============
            with tc.tile_pool(name="p3sb", bufs=2) as sp, \
                 tc.tile_pool(name="p3bl", bufs=3) as blp, \
                 tc.tile_pool(name="p3ps", bufs=2, space="PSUM") as pp, \
                 tc.tile_pool(name="p3env", bufs=1, space="PSUM") as envp, \
                 tc.tile_pool(name="p3ep", bufs=4, space="PSUM") as ep, \
                 tc.tile_pool(name="p3slab", bufs=1, space="PSUM") as slabp:
                for g in range(NG):
                    bk = g // GPB
                    sl = slice(g * GW, (g + 1) * GW)
                    b3 = blp.tile([128, B3W], BF, tag="b3")
                    nc.sync.dma_start(out=b3[:], in_=blob3_h[:, g, :])
                    tf0 = b3[:, B3_TF0:B3_TF0 + GW]
                    tf1 = b3[:, B3_TF1:B3_TF1 + GW]
                    t0b = b3[:, B3_T0B:B3_T0B + GW]
                    m2 = build_M2(sp, b3[:, B3_IDXM2:B3_IDXM2 + GW])
                    pe0 = ep.tile([128, GW], F32, tag="pe")
                    pe1 = ep.tile([128, GW], F32, tag="pe")
                    nc.tensor.matmul(out=pe0[:], lhsT=T1[:, bk, 0:128],
                                     rhs=m2[:], start=True, stop=True)
                    nc.tensor.matmul(out=pe1[:], lhsT=T1[:, bk, 128:256],
                                     rhs=m2[:], start=True, stop=True)
                    e0 = sp.tile([128, GW], BF, tag="e0")
                    e1 = sp.tile([128, GW], BF, tag="e1")
                    nc.vector.tensor_copy(out=e0[:], in_=pe0[:])
                    nc.scalar.activation(out=e1[:], in_=pe1[:], func=COPY)
                    hi = sp.tile([128, GW], BF, tag="hi")
                    nc.vector.tensor_copy(out=hi[0:64, :], in_=e0[0:64, :])
                    nc.vector.tensor_copy(out=hi[64:128, :], in_=e0[0:64, :])
                    # o0 = redmat-reduce of (wa*tf)*env
                    ma = sp.tile([128, GW], BF, tag="ma")
                    mb = sp.tile([128, GW], BF, tag="mb")
                    nc.vector.scalar_tensor_tensor(
                        out=ma[:], in0=tf0, scalar=wa[:, 0:1], in1=e0[:],
                        op0=MUL, op1=MUL)
                    nc.vector.scalar_tensor_tensor(
                        out=mb[:], in0=tf1, scalar=wa[:, 1:2], in1=e1[:],
                        op0=MUL, op1=MUL)
                    po0 = ep.tile([64, GW], F32, tag="pe")
                    nc.tensor.matmul(out=po0[:], lhsT=redmat[:], rhs=ma[:],
                                     start=True, stop=False)
                    nc.tensor.matmul(out=po0[:], lhsT=redmat[:], rhs=mb[:],
                                     start=False, stop=True)
                    nc.scalar.activation(out=q0[0:64, sl], in_=po0[:],
                                         func=COPY)
                    # ov_k = (wb*tf)*sa + (wt0*t0)*va_k
                    pa0 = sp.tile([128, GW], BF, tag="pa0")
                    pa1 = sp.tile([128, GW], BF, tag="pa1")
                    pb0 = sp.tile([128, GW], BF, tag="pb0")
                    pb1 = sp.tile([128, GW], BF, tag="pb1")
                    nc.vector.scalar_tensor_tensor(
                        out=pa0[:], in0=tf0, scalar=wb[:, 0:1], in1=hi[:],
                        op0=MUL, op1=MUL)
                    nc.vector.scalar_tensor_tensor(
                        out=pa1[:], in0=tf1, scalar=wb[:, 1:2], in1=hi[:],
                        op0=MUL, op1=MUL)
                    nc.vector.scalar_tensor_tensor(
                        out=pb0[:], in0=t0b, scalar=wt0[:, 0:1], in1=e0[:],
                        op0=MUL, op1=MUL)
                    nc.vector.scalar_tensor_tensor(
                        out=pb1[:], in0=t0b, scalar=wt0[:, 0:1], in1=e1[:],
                        op0=MUL, op1=MUL)
                    nc.vector.tensor_add(out=q0[64:128, sl], in0=pa0[64:128, :],
                                         in1=pb0[64:128, :])
                    nc.vector.tensor_add(out=q1[0:64, sl], in0=pa1[0:64, :],
                                         in1=pb1[0:64, :])
                    nc.vector.tensor_add(out=q1[64:128, sl], in0=pa1[64:128, :],
                                         in1=pb1[64:128, :])
                    # mlp0
                    a0t = sp.tile([128, GW], BF, tag="a0r")
                    nc.sync.dma_start(out=a0t[:], in_=acc0T[:, sl])
                    h1a = sp.tile([128, GW], BF, tag="h1a")
                    h1b = sp.tile([128, GW], BF, tag="h1b")
                    for mbi, hout in enumerate([h1a, h1b]):
                        msl = slice(mbi * 128, (mbi + 1) * 128)
                        ph = pp.tile([128, GW], F32, tag="mm")
                        nc.tensor.matmul(out=ph[:], lhsT=l0w1a[:, msl],
                                         rhs=a0t[:], start=True, stop=False)
                        nc.tensor.matmul(out=ph[:], lhsT=l0w1b[:, msl],
                                         rhs=q0[0:64, sl], start=False, stop=True)
                        nc.scalar.activation(out=hout[:], in_=ph[:], func=SILU)
                    h2a = sp.tile([128, GW], BF, tag="h2a")
                    h2b = sp.tile([128, GW], BF, tag="h2b")
                    for mbi, hout in enumerate([h2a, h2b]):
                        msl = slice(mbi * 128, (mbi + 1) * 128)
                        ph = pp.tile([128, GW], F32, tag="mm")
                        nc.tensor.matmul(out=ph[:], lhsT=l0w2a[:, msl],
                                         rhs=h1a[:], start=True, stop=False)
                        nc.tensor.matmul(out=ph[:], lhsT=l0w2b[:, msl],
                                         rhs=h1b[:], start=False, stop=True)
                        nc.scalar.activation(out=hout[:], in_=ph[:], func=SILU)
                    # W3 acc1 part (cols 0:128)
                    pl_a = pp.tile([128, GW], F32, tag="mm")
                    nc.tensor.matmul(out=pl_a[:], lhsT=l0w3a[:, 0:128],
                                     rhs=h2a[:], start=True, stop=False)
                    nc.tensor.matmul(out=pl_a[:], lhsT=l0w3b[:, 0:128],
                                     rhs=h2b[:], start=False, stop=True)
                    a1t = sp.tile([128, GW], BF, tag="a1")
                    nc.scalar.activation(out=a1t[:], in_=pl_a[:], func=COPY)
                    nc.sync.dma_start(out=acc1T[:, sl], in_=a1t[:])
                    # W3 env part (cols 128:256), edge-major
                    envps = envp.tile([128, GRP, 128], F32, tag="env")
                    for t in range(GRP):
                        ts = slice(t * 128, (t + 1) * 128)
                        nc.tensor.matmul(out=envps[:, t, :], lhsT=h2a[:, ts],
                                         rhs=l0w3a[:, 128:256],
                                         start=True, stop=False)
                        nc.tensor.matmul(out=envps[:, t, :], lhsT=h2b[:, ts],
                                         rhs=l0w3b[:, 128:256],
                                         start=False, stop=True)
                    bt = b3[:, B3_BT:B3_BT + GRP * 4].rearrange(
                        "p (t c) -> p t c", t=GRP)
                    x2 = build_X(sp, envps, bt, "X2")
                    m = build_M(sp, b3[:, B3_IDXM:B3_IDXM + GW])
                    if g % GPB == 0:
                        slabs2 = slabp.tile([128, 256], F32, tag="slab",
                                            name=f"slab2_{bk}")
                    for s in range(GRP):
                        nc.tensor.matmul(out=slabs2[:],
                                         lhsT=m[:, s, :], rhs=x2[:, s, :],
                                         start=(g % GPB == 0 and s == 0),
                                         stop=(g % GPB == GPB - 1 and s == GRP - 1))
                    if g % GPB == GPB - 1:
                        nc.scalar.activation(out=T2[:, bk, :], in_=slabs2[:],
                                             func=COPY)

            # =================== Phase 5: gather-2, TP1, mlp1 ================
            with tc.tile_pool(name="p5sb", bufs=2) as sp, \
                 tc.tile_pool(name="p5ps", bufs=2, space="PSUM") as pp, \
                 tc.tile_pool(name="p5ep", bufs=4, space="PSUM") as ep:
                for g in range(NG):
                    bk = g // GPB
                    sl = slice(g * GW, (g + 1) * GW)
                    b5 = sp.tile([128, B5W], BF, tag="b5")
                    nc.sync.dma_start(out=b5[:], in_=blob5_h[:, g, :])
                    a0t = sp.tile([128, GW], BF, tag="a0r")
                    nc.sync.dma_start(out=a0t[:], in_=acc0T[:, sl])
                    m2 = build_M2(sp, b5[:, 0:GW])
                    pe0 = ep.tile([128, GW], F32, tag="pe")
                    pe1 = ep.tile([128, GW], F32, tag="pe")
                    nc.tensor.matmul(out=pe0[:], lhsT=T2[:, bk, 0:128],
                                     rhs=m2[:], start=True, stop=True)
                    nc.tensor.matmul(out=pe1[:], lhsT=T2[:, bk, 128:256],
                                     rhs=m2[:], start=True, stop=True)
                    # o_last = redmat-reduce of env2 . q   (wx folded in W3 env)
                    pa = sp.tile([128, GW], BF, tag="pa")
                    pb = sp.tile([128, GW], BF, tag="pb")
                    nc.vector.tensor_tensor(out=pa[:], in0=pe0[:],
                                            in1=q0[:, sl], op=MUL)
                    nc.vector.tensor_tensor(out=pb[:], in0=pe1[:],
                                            in1=q1[:, sl], op=MUL)
                    pol = ep.tile([64, GW], F32, tag="pe")
                    nc.tensor.matmul(out=pol[:], lhsT=redmat[:], rhs=pa[:],
                                     start=True, stop=False)
                    nc.tensor.matmul(out=pol[:], lhsT=redmat[:], rhs=pb[:],
                                     start=False, stop=True)
                    ol = sp.tile([64, GW], BF, tag="ol")
                    nc.scalar.activation(out=ol[:], in_=pol[:], func=COPY)
                    # mlp1 (paired hidden chunks -> one SILU per layer)
                    a1t = sp.tile([128, GW], BF, tag="a1r")
                    nc.sync.dma_start(out=a1t[:], in_=acc1T[:, sl])
                    h1p = pp.tile([128, 2, GW], F32, tag="mmp")
                    for mbi in range(2):
                        msl = slice(mbi * 128, (mbi + 1) * 128)
                        nc.tensor.matmul(out=h1p[:, mbi, :], lhsT=l1w1a[:, msl],
                                         rhs=a0t[:], start=True, stop=False)
                        nc.tensor.matmul(out=h1p[:, mbi, :], lhsT=l1w1b[:, msl],
                                         rhs=a1t[:], start=False, stop=False)
                        nc.tensor.matmul(out=h1p[:, mbi, :], lhsT=l1w1c[:, msl],
                                         rhs=ol[:], start=False, stop=True)
                    h1 = sp.tile([128, 2 * GW], BF, tag="h1")
                    nc.scalar.activation(
                        out=h1[:], in_=h1p[:].rearrange("p c e -> p (c e)"),
                        func=SILU)
                    h2p = pp.tile([128, 2, GW], F32, tag="mmp")
                    for mbi in range(2):
                        msl = slice(mbi * 128, (mbi + 1) * 128)
                        nc.tensor.matmul(out=h2p[:, mbi, :], lhsT=l1w2a[:, msl],
                                         rhs=h1[:, 0:GW], start=True, stop=False)
                        nc.tensor.matmul(out=h2p[:, mbi, :], lhsT=l1w2b[:, msl],
                                         rhs=h1[:, GW:2 * GW],
                                         start=False, stop=True)
                    h2 = sp.tile([128, 2 * GW], BF, tag="h2")
                    nc.scalar.activation(
                        out=h2[:], in_=h2p[:].rearrange("p c e -> p (c e)"),
                        func=SILU)
                    ph = pp.tile([128, 2, GW], F32, tag="mmp")
                    nc.tensor.matmul(out=ph[:, 0, :], lhsT=l1w3a[:],
                                     rhs=h2[:, 0:GW], start=True, stop=False)
                    nc.tensor.matmul(out=ph[:, 0, :], lhsT=l1w3b[:],
                                     rhs=h2[:, GW:2 * GW], start=False, stop=True)
                    a2sb = sp.tile([128, GW], BF, tag="a2sb")
                    nc.vector.tensor_copy(out=a2sb[:], in_=ph[:, 0, :])
                    nc.sync.dma_start(out=acc2T[:, sl], in_=a2sb[:])

    nc.compile()
    return nc


# =====================================================================
# Host side
# =====================================================================


def _assign_bins(centers):
    """Greedy LPT: atoms -> 40 bins, <=128 atoms and <=4096 edges per bin."""
    deg = np.bincount(centers, minlength=N_ATOMS).astype(np.int64)
    order = np.argsort(-deg, kind="stable")
    nbins = N_CORES * N_BLK
    load = np.zeros(nbins, np.int64)
    count = np.zeros(nbins, np.int64)
    bin_of = np.empty(N_ATOMS, np.int64)
    cap = T_B * 128
    for a in order:
        masked = np.where(count < 128, load, 1 << 60)
        bi = int(np.argmin(masked))
        assert load[bi] + deg[a] <= cap, "bin overflow"
        bin_of[a] = bi
        load[bi] += deg[a]
        count[bi] += 1
    return bin_of


def _prep(edge_index, tensor_basis, tensor_features, scalar_embed):
    centers = np.asarray(edge_index[0])
    bin_of = _assign_bins(centers)
    core_of = bin_of // N_BLK
    blk_of = bin_of % N_BLK
    slot_of = np.empty(N_ATOMS, np.int64)
    for bi in range(N_CORES * N_BLK):
        atoms = np.where(bin_of == bi)[0]
        slot_of[atoms] = np.arange(len(atoms))

    tb = np.asarray(tensor_basis, np.float32)
    tf = np.asarray(tensor_features, np.float32)
    se = np.asarray(scalar_embed, np.float32)

    ecore = core_of[centers]
    ekey = blk_of[centers] * 128 + slot_of[centers]

    maps, perms = [], []
    for c in range(N_CORES):
        eids = np.where(ecore == c)[0]
        ek = ekey[eids]
        order = np.argsort(ek, kind="stable")
        eids = eids[order]
        ek = ek[order]
        perm = np.full(E_PAD, -1, np.int64)
        idxrel = np.full(E_PAD, -1.0, np.float32)
        for bl in range(N_BLK):
            run = eids[(ek // 128) == bl]
            n = len(run)
            assert n <= T_B * 128, f"block overflow {n}"
            base = bl * T_B * 128
            perm[base:base + n] = run
            idxrel[base:base + n] = (ekey[run] % 128).astype(np.float32)
        valid = perm >= 0
        psafe = np.where(valid, perm, 0)

        se_pad = (se[psafe] * valid[:, None]).astype(np.float32)
        tf_pad = (tf[psafe] * valid[:, None, None]).astype(np.float32)
        tb_pad = (tb[psafe] * valid[:, None]).astype(np.float32)

        # feature-major tf rows: f = comp*64 + chan
        tf_cm = np.ascontiguousarray(
            tf_pad.transpose(2, 1, 0)).reshape(256, E_PAD)
        tf0 = tf_cm[0:128]                     # [t0 | tv0]
        tf1 = tf_cm[128:256]                   # [tv1 | tv2]
        t0b = np.tile(tf_cm[0:64], (2, 1))     # [t0 | t0]
        idxrel_col = np.ascontiguousarray(idxrel.reshape(NT, 128).T)
        idxM = np.repeat(
            idxrel_col[:, :, None], 128, axis=2).reshape(128, E_PAD)
        idxM2 = np.tile(idxrel[None, :], (128, 1))
        seT = np.zeros((128, E_PAD), np.float32)
        seT[0:64] = se_pad.T
        basis_pm = np.ascontiguousarray(
            tb_pad.reshape(NT, 128, 4).transpose(1, 0, 2))  # [128, NT, 4]
        bt_g = basis_pm.reshape(128, NG, GRP * 4)

        b1 = np.empty((128, NG, B1W), np.float32)
        b1[:, :, B1_SE:B1_SE + GW] = seT.reshape(128, NG, GW)
        b1[:, :, B1_IDXM:B1_IDXM + GW] = idxM.reshape(128, NG, GW)
        b1[:, :, B1_BT:B1_BT + GRP * 4] = bt_g
        b3 = np.empty((128, NG, B3W), np.float32)
        b3[:, :, B3_TF0:B3_TF0 + GW] = tf0.reshape(128, NG, GW)
        b3[:, :, B3_TF1:B3_TF1 + GW] = tf1.reshape(128, NG, GW)
        b3[:, :, B3_T0B:B3_T0B + GW] = t0b.reshape(128, NG, GW)
        b3[:, :, B3_IDXM:B3_IDXM + GW] = idxM.reshape(128, NG, GW)
        b3[:, :, B3_IDXM2:B3_IDXM2 + GW] = idxM2.reshape(128, NG, GW)
        b3[:, :, B3_BT:B3_BT + GRP * 4] = bt_g
        b5 = idxM2.reshape(128, NG, GW)

        maps.append({"blob1": b1.astype(BF_NP), "blob3": b3.astype(BF_NP),
                     "blob5": np.ascontiguousarray(b5).astype(BF_NP)})
        perms.append(perm)
    return maps, perms


def _weights_maps(W_proj, w_tp0, w_tp1, l0, l1):
    bf = lambda a: np.ascontiguousarray(a).astype(BF_NP)
    wa = np.concatenate([w_tp0[0] * NORM,
                         np.tile(w_tp0[1] * NORM * INV_SQRT3, 3)])  # [256]
    wb = np.concatenate([w_tp0[3] * NORM,
                         np.tile(w_tp0[2] * NORM, 3)])              # [256]
    wx = np.concatenate([w_tp1[0] * NORM,
                         w_tp1[1] * NORM * INV_SQRT3])              # [128]
    wt0 = np.tile(w_tp0[3] * NORM, 2)                               # [128]
    l0w1, l0w2, l0w3 = l0
    l0w3 = l0w3.copy()
    l0w3[:, 128:256] = l0w3[:, 128:256] * wx[None, :]
    l1w1, l1w2, l1w3 = l1
    return {
        "iota_part": bf(np.repeat(
            np.arange(128, dtype=np.float32)[:, None], GW, 1)),
        "iota_tile": bf(np.tile(
            np.arange(128, dtype=np.float32)[None, :], (128, GRP))),
        "redmat": bf(np.tile(np.eye(64, dtype=np.float32), (2, 1))),
        "wa": np.ascontiguousarray(wa.reshape(2, 128).T, np.float32),
        "wb": np.ascontiguousarray(wb.reshape(2, 128).T, np.float32),
        "wt0": np.ascontiguousarray(wt0[:, None], np.float32),
        "wproj": bf(W_proj),
        "l0w1a": bf(l0w1[0:128]), "l0w1b": bf(l0w1[128:192]),
        "l0w2a": bf(l0w2[0:128]), "l0w2b": bf(l0w2[128:256]),
        "l0w3a": bf(l0w3[0:128]), "l0w3b": bf(l0w3[128:256]),
        "l1w1a": bf(l1w1[0:128]), "l1w1b": bf(l1w1[128:256]),
        "l1w1c": bf(l1w1[256:320]),
        "l1w2a": bf(l1w2[0:128]), "l1w2b": bf(l1w2[128:256]),
        "l1w3a": bf(l1w3[0:128]), "l1w3b": bf(l1w3[128:256]),
    }


_CACHE = {}


def kernel(edge_index, num_atoms, tensor_basis, tensor_features, scalar_embed,
           W_proj, w_tp0, w_tp1,
           lat0_W1, lat0_W2, lat0_W3, lat1_W1, lat1_W2, lat1_W3,
           _trace=False):
    if "nc" not in _CACHE:
        _CACHE["nc"] = build_graph()
    nc = _CACHE["nc"]

    wmaps = _weights_maps(
        np.asarray(W_proj, np.float32), np.asarray(w_tp0, np.float32),
        np.asarray(w_tp1, np.float32),
        (np.asarray(lat0_W1, np.float32), np.asarray(lat0_W2, np.float32),
         np.asarray(lat0_W3, np.float32)),
        (np.asarray(lat1_W1, np.float32), np.asarray(lat1_W2, np.float32),
         np.asarray(lat1_W3, np.float32)))

    maps, perms = _prep(np.asarray(edge_index), tensor_basis,
                        tensor_features, scalar_embed)
    in_maps = []
    for c in range(N_CORES):
        m = dict(maps[c])
        m.update(wmaps)
        in_maps.append(m)

    res = run_bass_kernel_spmd(nc, in_maps, core_ids=list(range(N_CORES)),
                               trace=_trace)
    out = np.empty((E, NS * 3), np.float32)
    for c in range(N_CORES):
        r = res.results[c]
        op = np.concatenate(
            [np.asarray(r["acc0T"]).astype(np.float32).T,
             np.asarray(r["acc1T"]).astype(np.float32).T,
             np.asarray(r["acc2T"]).astype(np.float32).T], axis=1)
        perm = perms[c]
        valid = perm >= 0
        out[perm[valid]] = op[valid]
    if _trace:
        kernel.last_exec_time_ns = res.exec_time_ns
    return out
